# revision 2
# baseline (speedup 1.0000x reference)
"""Trainium2 Bass kernel for a 2-layer Elman RNN decoder (nn_DecoderRNN) — v3.

Math per step (B=64, H=4761, T=128):
    h0 = tanh(b0 + h0 @ W_hh0.T)              b0 = b_ih0 + b_hh0 (input is zeros)
    h1 = tanh(b1 + h0 @ W_ih1.T + h1 @ W_hh1.T)
Output: stacked h1 over T steps, [B, T, H].

Strategy (8 NeuronCores, tensor-parallel over the output dim):
  - H padded to K=4864 = 8*608 = 128*38: core m owns output cols
    [608m, 608m+608); contraction runs over 38 uniform k-tiles of 128.
  - Contraction index is laid out p-major: PSUM-stationary slot (p, kt)
    holds h[38p + kt].  The gathered h [K,2,B] then reloads into SBUF with
    one contiguous 38*256B run per partition (single-descriptor DMA), and
    the weights are host-swizzled to the same order.
  - f32-ish precision via fp16 hi/lo splits: W*SW = Whi + Wlo, h = hhi + hlo
    (lo parts UNSCALED - fp16 exponent range covers them; scaling by 2^12
    is numerically identical, so we skip it).  The stationary packs
    [hhi | hlo] into 128 cols; the Whi stream accumulates hhi.Whi (psum
    rows 0:64) + hlo.Whi (rows 64:128); the Wlo stream ACCUMULATES INTO THE
    SAME PSUM: rows 0:64 += hhi.Wlo, rows 64:128 += hlo.Wlo (the last term
    is ~2^-23 - negligible but also mathematically correct).  One psum
    bank group per layer; preact = rows 0:64 + rows 64:128 (one DVE add).
  - Bias folded into the matmul via contraction slot 4863 = (p=127,kt=37):
    stationary forced to (hi=1, lo=0) there; W row 4863 = bias*SW.
  - Whi (3 matrices) resident in SBUF; Wlo streamed from HBM each step in
    4-k-tile blocks on the ACT HWDGE ring (nc.scalar) so it never blocks
    the latency-critical SP-ring DMAs (gather bounce, stationary reload).
  - New h shard [64,608] is hi/lo split, transposed via PE identity-matmuls
    into [k, {hi,lo}, b] layout, AllGathered, and reloaded into the next
    stationary as 4 chunk tiles so the next weight-apply starts on chunk 0
    while chunks 1-3 are still landing.
  - Emission interleaves each layer's transpose/gather pipeline into the
    next weight-apply's matmul stream so PE never idles on the gather.
"""

import os
import numpy as np

import concourse.bass as bass
import concourse.bacc as bacc
import concourse.tile as tile
from concourse import mybir
from concourse.bass_utils import run_bass_kernel_spmd

H_REAL = 4761
K = 4864            # padded hidden size = 8*608 = 128*38
B = 64              # batch
T_FULL = 128        # time steps
NCORES = 8
J = K // NCORES     # 608 output cols per core
KT = 38             # contraction k-tiles, all full 128 rows
KB = 4              # k-tiles per W_lo streaming DMA block
SW = 32.0           # weight scale (power of 2)
F16 = mybir.dt.float16
F32 = mybir.dt.float32

_ALU = mybir.AluOpType
_LO_BLOCKS = [(k, min(KB, KT - k)) for k in range(0, KT, KB)]  # [(0,4)...(36,2)]
_CHUNKS = [(0, 10), (10, 10), (20, 10), (30, 8)]  # stationary reload chunks


def build(tc, outs, ins, T):
    nc = tc.nc
    ys = outs["ys"]

    import contextlib
    with contextlib.ExitStack() as ctx:
        wpool = ctx.enter_context(tc.tile_pool(name="wpool", bufs=1))
        hstpool = ctx.enter_context(tc.tile_pool(name="hstpool", bufs=2))
        cpool = ctx.enter_context(tc.tile_pool(name="cpool", bufs=1))
        lopool = ctx.enter_context(tc.tile_pool(name="lopool", bufs=3))
        fpool = ctx.enter_context(tc.tile_pool(name="fpool", bufs=1))
        hpool = ctx.enter_context(tc.tile_pool(name="hpool", bufs=1))
        tspool = ctx.enter_context(tc.tile_pool(name="tspool", bufs=1))
        pa_pool = ctx.enter_context(tc.tile_pool(name="papool", bufs=2, space="PSUM"))
        tp_pool = ctx.enter_context(tc.tile_pool(name="tppool", bufs=2, space="PSUM"))
        dpool = ctx.enter_context(tc.tile_pool(name="dpool", bufs=2, space="DRAM"))

        # ---- constants ----
        ident = cpool.tile([B, B], F16)
        nc.sync.dma_start(ident, ins["ident"])

        # ---- resident hi weights: host-swizzled to [128, KT*J], p-major ----
        w_hi_sb = {}
        for wname in ("w0", "w1h", "w1i"):
            wsb = wpool.tile([128, KT * J], F16, name=f"{wname}_hi_sb")
            nc.scalar.dma_start(wsb, ins[f"{wname}_hi"])
            w_hi_sb[wname] = wsb

        # ---- stationary hidden states: 4 chunk tiles of [128, nkt, 2, B] ----
        # chunk q slot (p, i) holds h[38p + kt0 + i]
        def new_hst(tag):
            return [hstpool.tile([128, nkt * 2 * B], F16,
                                 name=f"hst_{tag}c{q}", tag=f"{tag}c{q}")
                    for q, (kt0, nkt) in enumerate(_CHUNKS)]

        def load_hst(chunks, src):
            # src is [K, 2, B] DRAM, k = 38p + kt (p-major).  The
            # bias-activation slot (p=127, kt=37) regenerates itself:
            # W[4863,4863] = 20*SW makes h[4863] = tanh(20) = 1.0 exactly.
            srcv = src.rearrange("(p k) s b -> p k s b", k=KT)
            for q, (kt0, nkt) in enumerate(_CHUNKS):
                nc.sync.dma_start(
                    chunks[q].rearrange("p (k s b) -> p k s b", s=2, b=B),
                    srcv[:, kt0:kt0 + nkt])

        h0st = new_hst("h0")
        h1st = new_hst("h1")
        load_hst(h0st, ins["h0t"])   # host pre-sets the ones slot
        load_hst(h1st, ins["h1t"])

        # ---- one weight apply over a subset of lo-blocks ----
        # All 4 matmuls per k-tile accumulate into one psum group:
        #   rows 0:64 = hhi.Whi + hhi.Wlo ; rows 64:128 = hlo.Whi + hlo.Wlo
        def apply_weight(wname, hst, psA, blocks, grp_start, grp_stop):
            whi = w_hi_sb[wname]
            lo_dram = ins[f"{wname}_lo"]
            for kt0, nkt in blocks:
                lo_t = lopool.tile([128, KB, J], F16, name="lo_t", tag="lo")
                nc.scalar.dma_start(
                    lo_t[:, 0:nkt, :],
                    lo_dram[:, kt0 * J:(kt0 + nkt) * J].rearrange(
                        "p (k j) -> p k j", j=J))
                for i in range(nkt):
                    kt = kt0 + i
                    q = min(kt // 10, 3)
                    cq0 = _CHUNKS[q][0]
                    st = hst[q][:, (kt - cq0) * 2 * B:(kt - cq0 + 1) * 2 * B]
                    first = grp_start and kt == 0
                    last = grp_stop and kt == KT - 1
                    for c0, c1 in ((0, 512), (512, J)):
                        nc.tensor.matmul(
                            psA[:, c0:c1], st, whi[:, kt * J + c0:kt * J + c1],
                            start=first, stop=False, skip_group_check=True)
                    for c0, c1 in ((0, 512), (512, J)):
                        nc.tensor.matmul(
                            psA[:, c0:c1], st, lo_t[:, i, c0:c1],
                            start=False, stop=last, skip_group_check=True)

        # ---- fold psum -> preact -> tanh -> hi/lo split ----
        def fold_tanh(psA):
            # DVE can read only one PSUM operand per instruction: stage the
            # hlo-contribution rows through SBUF, then add the main rows.
            xb = fpool.tile([B, J], F32, name="xb", tag="fxb")
            nc.vector.tensor_copy(xb, psA[B:128, 0:J])
            pre = fpool.tile([B, J], F32, name="pre", tag="fpre")
            nc.vector.tensor_tensor(pre, psA[0:B, 0:J], xb, _ALU.add)
            h_f32 = hpool.tile([B, J], F32, name="h_f32", tag="hf32")
            nc.scalar.activation(h_f32, pre,
                                 mybir.ActivationFunctionType.Tanh,
                                 bias=0.0, scale=1.0 / SW)
            return h_f32

        def split(h_f32):
            h_hi = hpool.tile([B, J], F16, name="h_hi", tag="hhi")
            nc.vector.tensor_copy(h_hi, h_f32)
            h_lo = hpool.tile([B, J], F16, name="h_lo", tag="hlo")
            nc.vector.tensor_tensor(h_lo, h_f32, h_hi, _ALU.subtract)
            return h_hi, h_lo

        # ---- transpose via PE identity matmul, stage interleaved [k, s, b] ----
        def transposes(h_hi, h_lo, tag):
            hts = tspool.tile([128, 5, 2, B], F16, name="hts", tag=f"ts{tag}")
            for s, src in ((0, h_hi), (1, h_lo)):
                for c in range(5):
                    cw = 128 if c < 4 else J - 512
                    tp = tp_pool.tile([128, B], F32, name="tp", tag="tp")
                    nc.tensor.matmul(tp[0:cw, :], src[:, 128 * c:128 * c + cw],
                                     ident, start=True, stop=True,
                                     skip_group_check=True)
                    nc.vector.tensor_copy(hts[0:cw, c, s, :], tp[0:cw, :])
            return hts

        def gather(hts, tag):
            agin = dpool.tile([J, 2, B], F16, name="agin", tag=f"agin{tag}")
            nc.sync.dma_start(
                agin[0:512, :, :].rearrange("(c p) s b -> p c s b", p=128),
                hts[:, 0:4, :, :])
            nc.sync.dma_start(agin[512:J, :, :], hts[0:J - 512, 4, :, :])
            agout = dpool.tile([K, 2, B], F16, name="agout", tag=f"agout{tag}",
                               addr_space="Shared")
            nc.gpsimd.collective_compute(
                "AllGather", _ALU.bypass,
                replica_groups=[list(range(NCORES))],
                ins=[agin.opt()], outs=[agout.opt()])
            return agout

        def reload_hst(tag, agout):
            chunks = new_hst(tag)
            load_hst(chunks, agout)
            return chunks

        # ---- time loop (unrolled; collectives can't live in HW loops) ----
        h1_parts = None
        for t in range(T):
            # layer 0
            psA0 = pa_pool.tile([128, 2 * J], F32, name="psA0", tag="pa")
            apply_weight("w0", h0st, psA0, _LO_BLOCKS[:1], True, False)
            if h1_parts is not None:
                # previous step's h1: transpose + gather + reload, hidden
                # behind this step's layer-0 matmul stream
                hts1 = transposes(*h1_parts, 1)
                h1st = reload_hst("h1", gather(hts1, 1))
            apply_weight("w0", h0st, psA0, _LO_BLOCKS[1:], False, True)
            h0_f32 = fold_tanh(psA0)
            h0_hi, h0_lo = split(h0_f32)

            # layer 1: W_hh1 (old h1) first to cover the h0 all-gather
            psA1 = pa_pool.tile([128, 2 * J], F32, name="psA1", tag="pa")
            apply_weight("w1h", h1st, psA1, _LO_BLOCKS[:1], True, False)
            hts0 = transposes(h0_hi, h0_lo, 0)
            h0st = reload_hst("h0", gather(hts0, 0))
            apply_weight("w1h", h1st, psA1, _LO_BLOCKS[1:], False, False)
            apply_weight("w1i", h0st, psA1, _LO_BLOCKS, False, True)
            h1_f32 = fold_tanh(psA1)
            nc.sync.dma_start(ys[0:B, t, 0:J], h1_f32)
            h1_parts = split(h1_f32) if t < T - 1 else None


# ------------------------------------------------------------------
# host side
# ------------------------------------------------------------------

def _pad_to(x, n, axis):
    w = [(0, 0)] * x.ndim
    w[axis] = (0, n - x.shape[axis])
    return np.pad(x, w)


def _swizzle(a):
    """[K, Jc] -> [128, KT*Jc]: contraction row 38*p+kt lands at
    [p, kt*Jc:(kt+1)*Jc] (p-major)."""
    Jc = a.shape[1]
    return np.ascontiguousarray(a.reshape(128, KT, Jc).reshape(128, KT * Jc))


def prep_inputs(hidden, W_ih0, W_hh0, b_ih0, b_hh0, W_ih1, W_hh1, b_ih1, b_hh1):
    f32 = np.float32

    def wsplit(W, bias):
        WT = _pad_to(_pad_to(np.asarray(W, f32).T, K, 0), K, 1) * f32(SW)
        if bias is not None:
            WT[K - 1, :] = _pad_to(np.asarray(bias, f32), K, 0) * f32(SW)
            # self-sustaining ones slot: h[K-1] = tanh(20) == 1.0f each step
            WT[K - 1, K - 1] = f32(20.0 * SW)
        hi = WT.astype(np.float16)
        lo = (WT - hi.astype(f32)).astype(np.float16)
        return hi, lo

    w0_hi, w0_lo = wsplit(W_hh0, np.asarray(b_ih0, f32) + np.asarray(b_hh0, f32))
    w1i_hi, w1i_lo = wsplit(W_ih1, None)
    w1h_hi, w1h_lo = wsplit(W_hh1, np.asarray(b_ih1, f32) + np.asarray(b_hh1, f32))

    def hprep(h):
        hT = _pad_to(np.asarray(h, f32), K, 1).T.copy()   # [K, B]
        hi = hT.astype(np.float16)
        lo = (hT - hi.astype(f32)).astype(np.float16)
        arr = np.zeros((K, 2, B), np.float16)
        arr[:, 0, :] = hi
        arr[:, 1, :] = lo
        arr[K - 1, 0, :] = 1.0                             # bias-activation slot
        arr[K - 1, 1, :] = 0.0
        return arr

    h0t = hprep(hidden[0])
    h1t = hprep(hidden[1])
    ident = np.eye(B, dtype=np.float16)

    in_maps = []
    for m in range(NCORES):
        js = slice(J * m, J * (m + 1))
        in_maps.append({
            "w0_hi": _swizzle(w0_hi[:, js]), "w0_lo": _swizzle(w0_lo[:, js]),
            "w1i_hi": _swizzle(w1i_hi[:, js]), "w1i_lo": _swizzle(w1i_lo[:, js]),
            "w1h_hi": _swizzle(w1h_hi[:, js]), "w1h_lo": _swizzle(w1h_lo[:, js]),
            "h0t": h0t, "h1t": h1t,
            "ident": ident,
        })
    return in_maps


_IN_SPECS = [
    ("w0_hi", [128, KT * J], np.float16), ("w0_lo", [128, KT * J], np.float16),
    ("w1i_hi", [128, KT * J], np.float16), ("w1i_lo", [128, KT * J], np.float16),
    ("w1h_hi", [128, KT * J], np.float16), ("w1h_lo", [128, KT * J], np.float16),
    ("h0t", [K, 2, B], np.float16), ("h1t", [K, 2, B], np.float16),
    ("ident", [B, B], np.float16),
]

_BUILD_CACHE = {}


def build_nc(T):
    if T in _BUILD_CACHE:
        return _BUILD_CACHE[T]
    nc = bacc.Bacc("TRN2", target_bir_lowering=False, debug=False,
                   num_devices=NCORES)
    ins = {name: nc.dram_tensor(name, shape, mybir.dt.from_np(np.dtype(dt)),
                                kind="ExternalInput").ap()
           for name, shape, dt in _IN_SPECS}
    outs = {"ys": nc.dram_tensor("ys", [B, T, J], mybir.dt.float32,
                                 kind="ExternalOutput").ap()}
    with tile.TileContext(nc) as tc:
        build(tc, outs, ins, T)
    nc.compile()
    _BUILD_CACHE[T] = nc
    return nc


def kernel(**inputs):
    inputs = {k: np.asarray(v) for k, v in inputs.items()}
    in_maps = prep_inputs(**inputs)
    nc = build_nc(T_FULL)
    trace = bool(int(os.environ.get("BASS_PROFILE", "0")))
    res = run_bass_kernel_spmd(nc, in_maps, core_ids=list(range(NCORES)),
                               trace=trace)
    kernel._last = res
    ys = np.concatenate([res.results[m]["ys"] for m in range(NCORES)], axis=2)
    return np.ascontiguousarray(ys[:, :, :H_REAL]).astype(np.float32)
